# revision 10
# baseline (speedup 1.0000x reference)
"""Trainium2 Bass kernel for nn_CrossAttention.

Problem: B=4, S=2048, D=512 cross-attention with 3 input streams:
  Qi, Ki, Vi = xi@Wq+bq, xi@Wk+bk, xi@Wv+bv   (i = 1..3)
  fused_xi = sum over j != i of softmax(Qi Kj^T / sqrt(512)) @ Vj
  out = concat(fused_x1..3, -1) @ Wo + bo

Sharding: 8 cores = (batch b in 0..3) x (query half in 0..1). Each core runs
an identical single-core program on its own data slice: full context for its
batch, a 1024-row query block, and replicated weights.

Key algebraic restructure (vs the naive Q/K path): since
  Qi Kj^T = (xi Wq + bq)(xj Wk + bk)^T
          = xi (Wq Wk^T) xj^T  +  [row-const]  +  bq (Wk^T xj^T)  +  [const]
and softmax is invariant to per-query-row constants, the whole K projection
disappears.  The host precomputes M = Wq Wk^T (512x512, trivial) and the
per-key bias t_j = scale * x_j (Wk bq); the device computes
  GT_i[b, q] = (M^T xi^T)[b, q]        (one projection per stream, own queries)
  S^T[k, q]  = sum_b xTj[b, k] GT_i[b, q]   (lhsT = raw resident xTj chunks!)
  w^T        = exp(scale*S^T + t_j[k])      (bias folded into the activation)
This removes all K-projection matmuls (192 per core) and the q-side biases.

Per-core algorithm (transposed layout, no transposes materialized):
  X^T [din, s] resident per context j; V[s, h] = X^T^T Wv (no bias; bv folded
  into bo' = bo + 2*sum_i bv@Wo_i since softmax rows sum to 1)
  S^T[k, q] = (X^T chunk)^T GT_i       (contract din)
  w^T       = exp(scale*S^T + t_j)
  O^T[h, q] = V^T w^T                  (contract k)
  z[q]      = sum_k w^T  (DVE partial sums + gpsimd partition all-reduce)
  out[q, :]+= (O^T chunk)^T @ Wo_i * (1/z)[q] per attention term; per-qt
  output DMA fires as soon as the last contribution lands.

Bulk matmuls run in bf16 (full PE rate) with fp32 PSUM accumulation; softmax
statistics and the final accumulation stay fp32.
"""

import numpy as np

B, S, DIN, DH, DOUT = 4, 2048, 512, 512, 512
P = 128
DC = DIN // P      # 4  din chunks (contraction chunks)
HT = DH // P       # 4  head tiles
ST = S // P        # 16 s tiles
KT = ST            # 16 k tiles
QW = 1024          # queries per core
QC = QW // 512     # 2  query chunks of 512
SCALE = 1.0 / float(np.sqrt(DH))

_CACHE = {}


def _build_program(loop_n=1):
    import contextlib

    import concourse.bacc as bacc
    import concourse.bass_isa as bass_isa
    import concourse.library_config as library_config
    import concourse.mybir as mybir
    import concourse.tile as tile

    dt = mybir.dt
    F32 = dt.float32
    BF16 = dt.bfloat16
    AF = mybir.ActivationFunctionType

    nc = bacc.Bacc("TRN2", target_bir_lowering=False, debug=False, num_devices=8)

    xT = [
        nc.dram_tensor(f"xT{i}", [DIN, S], BF16, kind="ExternalInput").ap()
        for i in range(3)
    ]
    Mt_d = nc.dram_tensor("Mt", [DIN, DIN], BF16, kind="ExternalInput").ap()
    Wv_d = nc.dram_tensor("Wv", [DIN, DH], BF16, kind="ExternalInput").ap()
    Wo_d = nc.dram_tensor("Wo", [3 * DH, DOUT], BF16, kind="ExternalInput").ap()
    bv_d = nc.dram_tensor("bv", [DH], BF16, kind="ExternalInput").ap()
    bo_d = nc.dram_tensor("bo", [DOUT], F32, kind="ExternalInput").ap()
    tj_d = nc.dram_tensor("tj", [3, S], F32, kind="ExternalInput").ap()
    out_d = nc.dram_tensor("out", [QW, DOUT], F32, kind="ExternalOutput").ap()

    def mm(out, lhsT, rhs, start, stop):
        assert lhsT.dtype == rhs.dtype, (lhsT.dtype, rhs.dtype)
        nc.tensor.matmul(out, lhsT, rhs, start=start, stop=stop)

    with tile.TileContext(nc) as tc:
        with (
            tc.tile_pool(name="const", bufs=1) as cpool,
            tc.tile_pool(name="xfull", bufs=2) as xfpool,
            tc.tile_pool(name="xq", bufs=1) as xqpool,
            tc.tile_pool(name="gt", bufs=1) as gtpool,
            tc.tile_pool(name="vsb", bufs=2) as vpool,
            tc.tile_pool(name="wts", bufs=3) as wtpool,
            tc.tile_pool(name="osb", bufs=2) as opool,
            tc.tile_pool(name="zps", bufs=2) as zppool,
            tc.tile_pool(name="accp", bufs=1) as accpool,
            tc.tile_pool(name="zsums", bufs=2) as zsumpool,
            tc.tile_pool(name="rbp", bufs=2) as rbpool,
            tc.tile_pool(name="fusedp", bufs=6) as fusedpool,
            tc.tile_pool(name="tmpf", bufs=2) as tmppool,
            tc.tile_pool(name="ps", bufs=4, space="PSUM") as pspool,
            tc.tile_pool(name="pso", bufs=1, space="PSUM") as psopool,
        ):
            # partition_all_reduce lives in the gpsimd "attn" ucode library
            nc.gpsimd.load_library(library_config.attn)

            # ---- constants (DMA emission order matters: the first PE work
            # needs m_sb + xq tiles, so those go first; wo is only needed by
            # the bias prologue / late epilogues, mid-kernel) ----
            m_sb = cpool.tile([P, DC, DIN], BF16, name="m_sb")
            wv_sb = cpool.tile([P, DC, DH], BF16, name="wv_sb")
            wo_sb = cpool.tile([P, 3 * HT, DOUT], BF16, name="wo_sb")
            bv2_sb = cpool.tile([P, HT], BF16, name="bv2_sb")
            bo_sb = cpool.tile([1, DOUT], F32, name="bo_sb")
            tj_sb = cpool.tile([P, 3 * KT], F32, name="tj_sb")
            ones_sb = cpool.tile([P, P], F32, name="ones_sb")
            bob_sb = cpool.tile([P, DOUT], F32, name="bob_sb")

            nc.sync.dma_start(out=m_sb[:], in_=Mt_d.rearrange("(c p) h -> p c h", p=P))

            loop_ctx = (
                tc.For_i(0, loop_n, 1) if loop_n > 1 else contextlib.nullcontext()
            )

            # ---- GT^T projection: gt[i][b, q] = sum_a M[a,b] xTi[a, q] ----
            def project_gt(i, src):
                g = gtpool.tile([P, DC, QW], BF16, name=f"gt_{i}")
                for qc in range(QC):
                    for bt in range(DC):
                        ps = pspool.tile([P, 512], F32, name="ps_g", tag="ps")
                        for dc in range(DC):
                            mm(
                                ps[:],
                                m_sb[:, dc, bt * P : (bt + 1) * P],
                                src[:, dc, qc * 512 : (qc + 1) * 512],
                                start=(dc == 0),
                                stop=(dc == DC - 1),
                            )
                        if bt % 2 == 0:
                            nc.scalar.activation(
                                g[:, bt, qc * 512 : (qc + 1) * 512], ps[:], AF.Copy
                            )
                        else:
                            nc.vector.tensor_copy(
                                g[:, bt, qc * 512 : (qc + 1) * 512], ps[:]
                            )
                return g

            # ---- V projection for context j (from the resident xT) ----
            def project_v(xf):
                v_sb = vpool.tile([P, ST, DH], BF16, name="v_sb")
                for st in range(ST):
                    ps = pspool.tile([P, 512], F32, name="ps_v", tag="ps")
                    for dc in range(DC):
                        mm(
                            ps[:],
                            xf[:, dc, st * P : (st + 1) * P],
                            wv_sb[:, dc, :],
                            start=(dc == 0),
                            stop=(dc == DC - 1),
                        )
                    # V without bias: bv is folded into bo'
                    if st % 2 == 0:
                        nc.vector.tensor_copy(v_sb[:, st, :], ps[:])
                    else:
                        nc.scalar.activation(v_sb[:, st, :], ps[:], AF.Copy)
                return v_sb

            # ---- attention units with a cross-unit software pipeline ----
            # One unit = (queries i vs context j) x one 512-query chunk.
            # The epilogue of unit u (PSUM->SBUF copies, z all-reduce +
            # reciprocal, normalized accumulation into fused_i, and after the
            # second pair of an i also the output projection) is emitted
            # interleaved into unit u+1's score phase so the PE never idles
            # waiting for ACT/DVE/GPSIMD epilogue work.
            fstate = {}

            def make_epilogue(i, qc, po, zp, pair_b, first_out, final_out, acc):
                state = {}

                def early():
                    # free the PV psum quickly (no data deps beyond po)
                    o_sb = opool.tile([P, HT, 512], BF16, name="o_sb")
                    for ht in range(HT):
                        if ht < 2:
                            nc.scalar.activation(
                                o_sb[:, ht, :], po[:, ht, :], AF.Copy
                            )
                        else:
                            nc.vector.tensor_copy(o_sb[:, ht, :], po[:, ht, :])
                    # z[q] broadcast across partitions via gpsimd all-reduce
                    zsum = zsumpool.tile([P, 512], F32, name="zsum")
                    nc.gpsimd.partition_all_reduce(
                        zsum[:], zp[:], P, bass_isa.ReduceOp.add
                    )
                    rb = rbpool.tile([P, 512], F32, name="rb")
                    nc.vector.reciprocal(rb[:], zsum[:])
                    if not pair_b:
                        fp = fusedpool.tile(
                            [P, HT, 512], BF16, name="fused", tag="fused"
                        )
                        for ht in range(HT):
                            nc.vector.tensor_mul(
                                fp[:, ht, :], o_sb[:, ht, :], rb[:]
                            )
                        fstate[(i, qc)] = fp
                    else:
                        fp = fstate.pop((i, qc))
                        tmp = tmppool.tile([P, HT, 512], BF16, name="tmpf")
                        for ht in range(HT):
                            nc.vector.tensor_mul(
                                tmp[:, ht, :], o_sb[:, ht, :], rb[:]
                            )
                        for ht in range(HT):
                            nc.vector.tensor_add(
                                fp[:, ht, :], tmp[:, ht, :], fp[:, ht, :]
                            )
                        state["fp"] = fp

                def late(qs):
                    fp = state["fp"]
                    qt = qc * 4 + qs
                    py = pspool.tile([P, 512], F32, name="ps_y", tag="ps")
                    for hc in range(HT):
                        mm(
                            py[:],
                            fp[:, hc, qs * P : (qs + 1) * P],
                            wo_sb[:, i * HT + hc, :],
                            start=(hc == 0),
                            stop=(hc == HT - 1),
                        )
                    base = bob_sb[:] if first_out else acc[:, qt, :]
                    nc.vector.tensor_add(acc[:, qt, :], py[:], base)
                    if final_out:
                        nc.sync.dma_start(
                            out=out_d.rearrange("(t p) d -> p t d", p=P)[:, qt, :],
                            in_=acc[:, qt, :],
                        )

                return {"early": early, "late": late if pair_b else None}

            def attn_unit(i, j, gt_i, xf, v_sb, qc, epi_args, acc, prev_epi):
                po = psopool.tile([P, HT, 512], F32, name="ps_o")
                zp = zppool.tile([P, 512], F32, name="zp")
                ps_s = {}

                def s_group(kt):
                    ps = pspool.tile([P, 512], F32, name="ps_s", tag="ps")
                    for dc in range(DC):
                        mm(
                            ps[:],
                            xf[:, dc, kt * P : (kt + 1) * P],
                            gt_i[:, dc, qc * 512 : (qc + 1) * 512],
                            start=(dc == 0),
                            stop=(dc == DC - 1),
                        )
                    ps_s[kt] = ps

                s_group(0)
                s_group(1)
                if prev_epi is not None:
                    prev_epi["early"]()
                for kt in range(KT):
                    if kt + 2 < KT:
                        s_group(kt + 2)
                    wt = wtpool.tile([P, 512], BF16, name="wt")
                    nc.scalar.activation(
                        wt[:],
                        ps_s.pop(kt)[:],
                        AF.Exp,
                        scale=SCALE,
                        bias=tj_sb[:, j * KT + kt : j * KT + kt + 1],
                    )
                    for ht in range(HT):
                        mm(
                            po[:, ht, :],
                            v_sb[:, kt, ht * P : (ht + 1) * P],
                            wt[:],
                            start=(kt == 0),
                            stop=(kt == KT - 1),
                        )
                    if kt == 0:
                        nc.vector.tensor_copy(zp[:], wt[:])
                    else:
                        nc.vector.tensor_add(zp[:], zp[:], wt[:])
                    if (
                        prev_epi is not None
                        and prev_epi["late"] is not None
                        and 5 <= kt <= 8
                    ):
                        prev_epi["late"](kt - 5)

                pair_b, first_out, final_out = epi_args
                return make_epilogue(
                    i, qc, po, zp, pair_b, first_out, final_out, acc
                )

            # ---- main schedule ----
            # loop_n > 1 repeats the whole body on-device (used only for
            # timing measurements; output is still written every iteration)
            with loop_ctx:
                acc = accpool.tile([P, QW // P, DOUT], F32, name="acc")

                # upfront query halves for the two streams whose GT is needed
                # before their own context is resident (j=0 pairs with 1, 2)
                xq = {}
                for i in (1, 2):
                    xq[i] = xqpool.tile([P, DC, QW], BF16, name=f"xq_{i}")
                    nc.sync.dma_start(
                        out=xq[i][:],
                        in_=xT[i][:, 0:QW].rearrange("(c p) s -> p c s", p=P),
                    )
                # first context resident tile
                xf0 = xfpool.tile([P, DC, S], BF16, name="xf", tag="xf")
                nc.sync.dma_start(
                    out=xf0[:], in_=xT[0].rearrange("(c p) s -> p c s", p=P)
                )
                # remaining constants (emitted after the startup-critical DMAs)
                nc.sync.dma_start(
                    out=wv_sb[:], in_=Wv_d.rearrange("(c p) h -> p c h", p=P)
                )
                nc.sync.dma_start(
                    out=tj_sb[:], in_=tj_d.rearrange("j (t p) -> p (j t)", p=P)
                )
                nc.sync.dma_start(
                    out=bv2_sb[:], in_=bv_d.rearrange("(t p) -> p t", p=P)
                )
                nc.sync.dma_start(
                    out=bo_sb[:], in_=bo_d.rearrange("(a d) -> a d", a=1)
                )
                nc.sync.dma_start(
                    out=wo_sb[:], in_=Wo_d.rearrange("(c p) h -> p c h", p=P)
                )
                nc.vector.memset(ones_sb[:], 1.0)

                gt = {}
                gt[1] = project_gt(1, xq[1])
                gt[2] = project_gt(2, xq[2])

                v0 = project_v(xf0)
                # gt[0] from the resident context tile: its query block is
                # the first QW columns, which is all project_gt touches
                gt[0] = project_gt(0, xf0)

                # bo' = bo + 2*sum_i bv @ Wo_i ; broadcast over partitions.
                # Emitted here so the PE only reaches it once wo has arrived.
                nc.vector.tensor_scalar_mul(bv2_sb[:], bv2_sb[:], 2.0)
                ps_bo = pspool.tile([1, DOUT], F32, name="ps_bo", tag="ps")
                n = 0
                for i in range(3):
                    for c in range(DC):
                        mm(
                            ps_bo[:],
                            bv2_sb[:, c : c + 1],
                            wo_sb[:, i * HT + c, :],
                            start=(n == 0),
                            stop=(n == 11),
                        )
                        n += 1
                bo1_sb = cpool.tile([1, DOUT], F32, name="bo1_sb")
                nc.vector.tensor_add(bo1_sb[:], ps_bo[:], bo_sb[:])
                ps_bob = pspool.tile([P, DOUT], F32, name="ps_bob", tag="ps")
                mm(ps_bob[:], ones_sb[0:1, :], bo1_sb[:], start=True, stop=True)
                nc.scalar.activation(bob_sb[:], ps_bob[:], AF.Copy)

                pending = None
                occur = {0: 0, 1: 0, 2: 0}
                xf_cur, v_cur = xf0, v0
                for jn, j in enumerate((0, 1, 2)):
                    if jn > 0:
                        v_cur = project_v(xf_cur)
                    # prefetch next context while this one computes
                    if jn < 2:
                        xf_next = xfpool.tile([P, DC, S], BF16, name="xf", tag="xf")
                        nc.sync.dma_start(
                            out=xf_next[:],
                            in_=xT[(j + 1)].rearrange("(c p) s -> p c s", p=P),
                        )
                    # pair_b streams first: frees their fused tiles before
                    # the pair_a stream allocates new ones (peak 5, not 6)
                    pairs = sorted(
                        (i for i in range(3) if i != j),
                        key=lambda i: (occur[i] == 0, i),
                    )
                    for i in pairs:
                        pair_b = occur[i] == 1
                        for qc in range(QC):
                            pending = attn_unit(
                                i, j, gt[i], xf_cur, v_cur, qc,
                                (pair_b, i == 2, i == 1), acc, pending,
                            )
                        occur[i] += 1
                    if jn < 2:
                        xf_cur = xf_next
                # flush the last unit's epilogue (its lates also fire the
                # final per-qt output DMAs)
                pending["early"]()
                for qs in range(4):
                    pending["late"](qs)

    nc.compile()
    return nc


def _get_program():
    if "nc" not in _CACHE:
        _CACHE["nc"] = _build_program()
    return _CACHE["nc"]


def _make_in_maps(inputs):
    import ml_dtypes

    bf16 = ml_dtypes.bfloat16

    x = [np.asarray(inputs[k], np.float64) for k in ("x1", "x2", "x3")]
    Wq = np.asarray(inputs["Wq"], np.float64)
    Wk = np.asarray(inputs["Wk"], np.float64)
    bq = np.asarray(inputs["bq"], np.float64)

    # host-fused score weights: M = Wq Wk^T; per-key bias t_j = scale*x_j(Wk bq)
    M = Wq @ Wk.T
    wkbq = Wk @ bq
    tj = np.stack(
        [xi @ wkbq for xi in x], axis=1
    )  # [B, 3, S]
    tj = (SCALE * tj).astype(np.float32)

    common = {"Mt": np.ascontiguousarray(M).astype(bf16)}
    for k in ("Wv", "Wo", "bv"):
        common[k] = np.ascontiguousarray(np.asarray(inputs[k], np.float32)).astype(
            bf16
        )
    common["bo"] = np.ascontiguousarray(np.asarray(inputs["bo"], np.float32))

    in_maps = []
    for b in range(B):
        xTb = [np.ascontiguousarray(xi[b].T).astype(bf16) for xi in x]  # [512, 2048]
        tjb = tj[b]  # [3, S]
        for half in range(2):
            if half == 0:
                perm = xTb
                tjp = tjb
            else:
                # query block must be the first 1024 columns; k-order is
                # irrelevant (softmax sums over k) but tj must follow it
                perm = [
                    np.ascontiguousarray(
                        np.concatenate([t[:, QW:], t[:, :QW]], axis=1)
                    )
                    for t in xTb
                ]
                tjp = np.ascontiguousarray(
                    np.concatenate([tjb[:, QW:], tjb[:, :QW]], axis=1)
                )
            m = dict(common)
            for i in range(3):
                m[f"xT{i}"] = perm[i]
            m["tj"] = np.ascontiguousarray(tjp, np.float32)
            in_maps.append(m)
    return in_maps


def kernel(**inputs):
    from concourse.bass_utils import run_bass_kernel_spmd

    nc = _get_program()
    in_maps = _make_in_maps(inputs)
    res = run_bass_kernel_spmd(nc, in_maps, core_ids=list(range(8)))
    _CACHE["last_results"] = res

    y = np.empty((B, S, DOUT), np.float32)
    for c, r in enumerate(res.results):
        b, half = divmod(c, 2)
        y[b, half * QW : (half + 1) * QW] = r["out"]
    return y
